# revision 15
# baseline (speedup 1.0000x reference)
"""Trainium2 Bass kernel for nn_Decoder_block (pyramid-attention decoder block).

Sharding: data-parallel over batch B=16 across 8 NeuronCores (2 batches/core),
parameters replicated, no collectives.

Device dataflow (per core, per batch; feature-major X^T for matmuls,
token-major only for LayerNorm):
  emb = gather(E_ex+E_res fused)[idx] + gather(E_cat)[idx] + pos  (indirect DMA)
  xhat1 = LN(emb) token-major (E[x^2]-mean^2 form, in place), PE-transpose ->
          ln1T fp32 + x1T bf16 feature-major
  attn1: Q^T/K^T feature-major, V token-major with a per-head ones column so
         sumexp falls out of the AV matmul; scores computed transposed
         [kpos, q] so no P transpose is ever needed; no max-subtraction
         (scores bounded); softmax 1/sumexp deferred to a per-head broadcast
         multiply before the Wo projection; V bias deferred via a K=1 rank-1
         matmul of bv x sumexp; pyramid causal masking via 3 shifted diagonal
         strips applied to boundary tiles only, fully-masked tiles skipped.
  res2T = ln1T*g1 + Wo(attn) + rank1(bo1+be1)
  attn2: queries from res2T (no LN), K/V from en = LN(en_out), pooled levels
         computed on the input (pooling commutes with projection).
  res3T -> transpose -> LN3 -> transpose -> FFN (relu fused in ACT copy) ->
  final^T = x3T*g3 + FFN + rank1(fb2+be3) -> transpose -> out

Host folding: LN gammas folded into following weights, betas into following
biases; 1/sqrt(dh) folded into Wq/bq. bf16 matmuls with fp32 accumulation
(validated vs fp32 reference: absmax err ~0.25% of output scale).
"""
import os
import numpy as np
import ml_dtypes

B, S, D, H = 16, 1024, 512, 8
NUM_Q, NUM_C = 10000, 300
dh = D // H
KT = S + S // 2 + S // 4   # 1792 pooled key positions
NKT = KT // 128            # 14
NCORES = 8
BP = B // NCORES
NT = S // 128              # 8
DT = D // 128              # 4
BF = ml_dtypes.bfloat16
F32 = np.float32

_prog_cache = {}


def _tile_status(kt, qh):
    if kt < 8:
        f, j0 = 1, kt * 128
    elif kt < 12:
        f, j0 = 2, (kt - 8) * 128
    else:
        f, j0 = 4, (kt - 12) * 128
    i_min, i_max = qh * 512, qh * 512 + 511
    if (j0 + 1) * f - 1 > i_max:
        return "skip"
    if (j0 + 128) * f - 1 <= i_min:
        return "full"
    return "mask"


STATUS = {(kt, qh): _tile_status(kt, qh) for kt in range(NKT) for qh in range(2)}


def _mask_slice(kt, qh):
    """boundary tile -> (strip_index, col_offset) into the 3 mask strips."""
    if kt < 8:
        return 0, 384 + 512 * qh - 128 * kt
    if kt < 12:
        return 1, 256 + 512 * qh - 256 * (kt - 8)
    return 2, 0


def _build_strips():
    p = np.arange(128)[:, None]
    x0 = np.arange(896)[None, :]
    m0 = np.where(x0 - 384 >= p, 0.0, -1e9)
    x1 = np.arange(768)[None, :]
    m1 = np.where(x1 - 256 >= 2 * p + 1, 0.0, -1e9)
    x2 = np.arange(512)[None, :]
    m2 = np.where(x2 >= 4 * p + 3, 0.0, -1e9)
    return [m.astype(F32) for m in (m0, m1, m2)]


def build_program():
    import concourse.bass as bass
    import concourse.mybir as mybir
    import concourse.tile as tile
    from concourse import bacc
    from concourse.masks import make_identity
    from contextlib import ExitStack

    dt = mybir.dt
    Alu = mybir.AluOpType
    Act = mybir.ActivationFunctionType

    nc = bacc.Bacc(None, target_bir_lowering=False)

    def din(name, shape, d=dt.bfloat16):
        return nc.dram_tensor(name, list(shape), d, kind="ExternalInput")

    idx_ex = din("idx_ex", [128, BP * NT], dt.int32)
    idx_cat = din("idx_cat", [128, BP * NT], dt.int32)
    pos = din("pos", [BP, S, D], dt.float32)
    en0 = din("en0", [BP, S, D], dt.float32)
    tab_ex = din("tab_ex", [2 * NUM_Q, D])
    tab_cat = din("tab_cat", [2 * NUM_C, D])
    mask0 = din("mask0", [128, 896])
    mask1 = din("mask1", [128, 768])
    mask2 = din("mask2", [128, 512])
    wnames = ["wq1", "wk1", "wv1", "wo1", "wq2", "wk2", "wv2", "wo2", "fw1", "fw2"]
    wdram = {n: din(n, [128, DT, D]) for n in wnames}
    bias_names = ["bq1", "bk1", "bq2", "bk2", "fb1", "g1c", "g3c"]
    bdram = {n: din(n, [128, DT], dt.float32) for n in bias_names}
    row_names = ["bv1", "bv2", "brow1", "brow2", "browF"]
    rdram = {n: din(n, [1, D]) for n in row_names}
    out = nc.dram_tensor("out", [BP, S, D], dt.float32, kind="ExternalOutput")
    DBG = bool(os.environ.get("KDEBUG"))
    dbg = {}
    if DBG:
        for n, shape, d in [
            ("d_ln1T", [128, DT, S], dt.float32),
            ("d_enT", [128, DT, S], dt.bfloat16),
            ("d_q1T", [128, DT, S], dt.bfloat16),
            ("d_k1T", [128, DT, KT], dt.bfloat16),
            ("d_v65", [128, NKT, 8 * 65], dt.bfloat16),
            ("d_woRhs", [128, DT, S], dt.bfloat16),
            ("d_res2T", [128, DT, S], dt.float32),
            ("d_res3T", [128, DT, S], dt.float32),
            ("d_x3T", [128, DT, S], dt.float32),
            ("d_hT", [128, DT, S], dt.bfloat16),
            ("d_finT", [128, DT, S], dt.float32),
        ]:
            dbg[n] = nc.dram_tensor(n, shape, d, kind="ExternalOutput")

    with tile.TileContext(nc) as tc, ExitStack() as ctx:
        cp = ctx.enter_context(tc.tile_pool(name="const", bufs=1))
        wp = ctx.enter_context(tc.tile_pool(name="weights", bufs=1))
        big = ctx.enter_context(tc.tile_pool(name="big", bufs=1))
        sp = ctx.enter_context(tc.tile_pool(name="scratch", bufs=3))
        ep = ctx.enter_context(tc.tile_pool(name="emb", bufs=3))
        pp = ctx.enter_context(tc.tile_pool(name="psum", bufs=2, space="PSUM"))
        pp_sc = ctx.enter_context(tc.tile_pool(name="psum_sc", bufs=2,
                                               space="PSUM"))
        pp_av = ctx.enter_context(tc.tile_pool(name="psum_av", bufs=2,
                                               space="PSUM"))
        pp_tr = ctx.enter_context(tc.tile_pool(name="psum_tr", bufs=2,
                                               space="PSUM"))

        # ---- constants / weights ----
        id_f = cp.tile([128, 128], dt.float32, tag="id_f", name="id_f")
        make_identity(nc, id_f[:])
        id_b = cp.tile([128, 128], dt.bfloat16, tag="id_b", name="id_b")
        make_identity(nc, id_b[:])
        ones_row = cp.tile([1, S], dt.bfloat16, tag="ones_row", name="ones_row")
        nc.vector.memset(ones_row[:], 1.0)
        eps_col = cp.tile([128, 1], dt.float32, tag="eps_col", name="eps_col")
        nc.vector.memset(eps_col[:], 1e-5)

        idx_ex_sb = cp.tile([128, BP * NT], dt.int32, tag="idx_ex", name="idx_ex")
        nc.sync.dma_start(out=idx_ex_sb[:], in_=idx_ex[:])
        idx_cat_sb = cp.tile([128, BP * NT], dt.int32, tag="idx_cat",
                             name="idx_cat")
        nc.sync.dma_start(out=idx_cat_sb[:], in_=idx_cat[:])

        w = {}
        for n in wnames:
            w[n] = wp.tile([128, DT, D], dt.bfloat16, tag=f"w_{n}", name=f"w_{n}")
            nc.sync.dma_start(out=w[n][:], in_=wdram[n][:])
        bias = {}
        for n in bias_names:
            bias[n] = cp.tile([128, DT], dt.float32, tag=f"b_{n}", name=f"b_{n}")
            nc.sync.dma_start(out=bias[n][:], in_=bdram[n][:])
        rows = {}
        for n in row_names:
            rows[n] = cp.tile([1, D], dt.bfloat16, tag=f"r_{n}", name=f"r_{n}")
            nc.sync.dma_start(out=rows[n][:], in_=rdram[n][:])
        strips = []
        for i, md in enumerate([mask0, mask1, mask2]):
            st = cp.tile(list(md.shape), dt.bfloat16, tag=f"strip{i}",
                         name=f"strip{i}")
            nc.sync.dma_start(out=st[:], in_=md[:])
            strips.append(st)

        # ---------- helpers ----------
        def ln_inplace(tiles):
            """LayerNorm each [128, D] fp32 tile over free dim, in place.
            var = E[x^2] - mean^2; xhat = x*rstd - mean*rstd."""
            nt = len(tiles)
            nm, ssq = [], []
            for t in range(nt):
                ns = sp.tile([128, 1], dt.float32, tag="ln_ns", name="ln_ns",
                             bufs=nt)
                nc.vector.tensor_reduce(out=ns[:], in_=tiles[t][:],
                                        axis=mybir.AxisListType.X, op=Alu.add,
                                        negate=True)
                sq = sp.tile([128, D], dt.float32, tag="ln_sq", name="ln_sq",
                             bufs=1)
                sacc = sp.tile([128, 1], dt.float32, tag="ln_sacc",
                               name="ln_sacc", bufs=nt)
                nc.scalar.activation(out=sq[:], in_=tiles[t][:],
                                     func=Act.Square, accum_out=sacc[:])
                nm.append(ns)
                ssq.append(sacc)
            for t in range(nt):
                nmm = sp.tile([128, 1], dt.float32, tag="ln_nm", name="ln_nm",
                              bufs=nt)
                nc.vector.tensor_scalar(out=nmm[:], in0=nm[t][:],
                                        scalar1=1.0 / D, scalar2=None,
                                        op0=Alu.mult)
                nm[t] = nmm
            var = []
            for t in range(nt):
                m2 = sp.tile([128, 1], dt.float32, tag="ln_m2", name="ln_m2")
                nc.vector.tensor_tensor(out=m2[:], in0=nm[t][:], in1=nm[t][:],
                                        op=Alu.mult)
                v = sp.tile([128, 1], dt.float32, tag="ln_var", name="ln_var",
                            bufs=nt)
                nc.vector.scalar_tensor_tensor(
                    out=v[:], in0=ssq[t][:], scalar=1.0 / D, in1=m2[:],
                    op0=Alu.mult, op1=Alu.subtract)
                var.append(v)
            std = []
            for t in range(nt):  # ACT phase: Sqrt
                s = sp.tile([128, 1], dt.float32, tag="ln_std", name="ln_std",
                            bufs=nt)
                nc.scalar.activation(out=s[:], in_=var[t][:], func=Act.Sqrt,
                                     scale=1.0, bias=eps_col[:])
                std.append(s)
            for t in range(nt):
                r = sp.tile([128, 1], dt.float32, tag="ln_rstd", name="ln_rstd")
                nc.vector.reciprocal(out=r[:], in_=std[t][:])
                mrs = sp.tile([128, 1], dt.float32, tag="ln_mrs", name="ln_mrs")
                nc.vector.tensor_tensor(out=mrs[:], in0=nm[t][:], in1=r[:],
                                        op=Alu.mult)
                nc.vector.tensor_scalar(out=tiles[t][:], in0=tiles[t][:],
                                        scalar1=r[:], scalar2=mrs[:],
                                        op0=Alu.mult, op1=Alu.add)

        def transpose_to_fm(src_tiles, dsts, t0=0):
            """PE-transpose token-major [128, D] tiles into feature-major
            [128, DT, *] destination tiles at col range t*128..(t+1)*128."""
            ident = id_f if src_tiles[0].dtype == dt.float32 else id_b
            for i, xt in enumerate(src_tiles):
                t = t0 + i
                for d_ in range(DT):
                    ptr = pp_tr.tile([128, 128], xt.dtype, tag="tr", name="tr")
                    nc.tensor.transpose(out=ptr[:],
                                        in_=xt[:, d_ * 128:(d_ + 1) * 128],
                                        identity=ident[:])
                    for dst in dsts:
                        nc.vector.tensor_copy(
                            out=dst[:, d_, t * 128:(t + 1) * 128], in_=ptr[:])

        def proj_fm(wtile, rhs, ncols, dst, dst_c0, bias_col=None,
                    func=Act.Identity, scale=1.0):
            for mt in range(DT):
                for c0 in range(0, ncols, 512):
                    nch = min(512, ncols - c0)
                    ps = pp.tile([128, 512], dt.float32, tag="proj", name="proj")
                    for kt in range(DT):
                        nc.tensor.matmul(
                            out=ps[:, :nch],
                            lhsT=wtile[:, kt, mt * 128:(mt + 1) * 128],
                            rhs=rhs[:, kt, c0:c0 + nch],
                            start=(kt == 0), stop=(kt == DT - 1))
                    if bias_col is not None:
                        nc.scalar.activation(
                            out=dst[:, mt, dst_c0 + c0:dst_c0 + c0 + nch],
                            in_=ps[:, :nch], func=func,
                            bias=bias_col[:, mt:mt + 1], scale=scale)
                    else:
                        nc.scalar.activation(
                            out=dst[:, mt, dst_c0 + c0:dst_c0 + c0 + nch],
                            in_=ps[:, :nch], func=Act.Copy, scale=scale)

        def pool_half(src, src_cols, dst):
            # sum-pool; the 1/2 (or 1/4) is folded into the projection
            # epilogue's ACT scale
            for kt in range(DT):
                v = src[:, kt, 0:src_cols].rearrange("p (c two) -> p c two",
                                                     two=2)
                nc.vector.tensor_tensor(
                    out=dst[:, kt, 0:src_cols // 2], in0=v[:, :, 0],
                    in1=v[:, :, 1], op=Alu.add)

        def vproj(wtile, srcs, v65):
            for mt in range(NKT):
                if mt < 8:
                    src, c0, scl = srcs[0], mt * 128, 1.0
                elif mt < 12:
                    src, c0, scl = srcs[1], (mt - 8) * 128, 0.5
                else:
                    src, c0, scl = srcs[2], (mt - 12) * 128, 0.25
                ps = pp.tile([128, 512], dt.float32, tag="proj", name="proj")
                for kt in range(DT):
                    nc.tensor.matmul(
                        out=ps[:], lhsT=src[:, kt, c0:c0 + 128],
                        rhs=wtile[:, kt, :], start=(kt == 0),
                        stop=(kt == DT - 1))
                vv = v65[:, mt, :].rearrange("p (h c) -> p h c", c=65)
                nc.scalar.activation(
                    out=vv[:, :, 0:64],
                    in_=ps[:].rearrange("p (h c) -> p h c", c=64),
                    func=Act.Copy, scale=scl)
                nc.vector.memset(vv[:, :, 64:65], 1.0)

        def attention(qT, kT, v65, bv_row, woRhs):
            for h in range(H):
                p0 = (h % 2) * 64
                hd = h // 2
                for qh in range(2):
                    av = pp_av.tile([65, 512], dt.float32, tag="av", name="av")
                    acts = [kt for kt in range(NKT)
                            if STATUS[(kt, qh)] != "skip"]
                    for i, kt in enumerate(acts):
                        ps = pp_sc.tile([128, 512], dt.float32, tag="sc",
                                        name="sc")
                        nc.tensor.matmul(
                            out=ps[:],
                            lhsT=kT[p0:p0 + 64, hd, kt * 128:(kt + 1) * 128],
                            rhs=qT[p0:p0 + 64, hd, qh * 512:(qh + 1) * 512],
                            start=True, stop=True)
                        if STATUS[(kt, qh)] == "mask":
                            si, off = _mask_slice(kt, qh)
                            nc.vector.tensor_tensor(
                                out=ps[:], in0=ps[:],
                                in1=strips[si][:, off:off + 512], op=Alu.add)
                        pexp = sp.tile([128, 512], dt.bfloat16, tag="pexp",
                                       name="pexp")
                        nc.scalar.activation(out=pexp[:], in_=ps[:],
                                             func=Act.Exp)
                        nc.tensor.matmul(
                            out=av[:], lhsT=v65[:, kt, h * 65:(h + 1) * 65],
                            rhs=pexp[:], start=(i == 0),
                            stop=(i == len(acts) - 1))
                    s_sb = sp.tile([1, 512], dt.float32, tag="s_sb", name="s_sb", bufs=2)
                    nc.vector.tensor_copy(out=s_sb[:], in_=av[64:65, :])
                    s_bf = sp.tile([1, 512], dt.bfloat16, tag="s_bf",
                                   name="s_bf", bufs=2)
                    nc.vector.tensor_copy(out=s_bf[:], in_=s_sb[:])
                    r = sp.tile([1, 512], dt.float32, tag="r_sb", name="r_sb", bufs=2)
                    nc.vector.reciprocal(out=r[:], in_=s_sb[:])
                    r_bf = sp.tile([1, 512], dt.bfloat16, tag="r_bf",
                                   name="r_bf", bufs=2)
                    nc.vector.tensor_copy(out=r_bf[:], in_=r[:])
                    nc.tensor.matmul(
                        out=av[0:64, :], lhsT=bv_row[0:1, h * 64:(h + 1) * 64],
                        rhs=s_bf[:], start=False, stop=True,
                        skip_group_check=True)
                    # broadcast r across 64 partitions via K=1 rank-1 matmul
                    rbc_ps = pp_sc.tile([64, 512], dt.float32, tag="sc",
                                        name="rbc_ps")
                    nc.tensor.matmul(out=rbc_ps[:],
                                     lhsT=ones_row[0:1, 0:64], rhs=r_bf[:],
                                     start=True, stop=True)
                    rbc = sp.tile([64, 512], dt.float32, tag="rbc", name="rbc",
                                  bufs=2)
                    nc.vector.tensor_copy(out=rbc[:], in_=rbc_ps[:])
                    nc.vector.tensor_tensor(
                        out=woRhs[p0:p0 + 64, hd, qh * 512:(qh + 1) * 512],
                        in0=av[0:64, :], in1=rbc[:], op=Alu.mult)

        def wo_proj_residual(wtile, woRhs, brow, resdst, ln_src, g_col):
            for mt in range(DT):
                for c0 in range(0, S, 512):
                    ps = pp.tile([128, 512], dt.float32, tag="proj", name="proj")
                    for kt in range(DT):
                        nc.tensor.matmul(
                            out=ps[:],
                            lhsT=wtile[:, kt, mt * 128:(mt + 1) * 128],
                            rhs=woRhs[:, kt, c0:c0 + 512],
                            start=(kt == 0), stop=False)
                    nc.tensor.matmul(
                        out=ps[:], lhsT=brow[0:1, mt * 128:(mt + 1) * 128],
                        rhs=ones_row[0:1, c0:c0 + 512], start=False, stop=True)
                    if g_col is not None:
                        nc.vector.scalar_tensor_tensor(
                            out=resdst[:, mt, c0:c0 + 512],
                            in0=ln_src[:, mt, c0:c0 + 512],
                            scalar=g_col[:, mt:mt + 1], in1=ps[:],
                            op0=Alu.mult, op1=Alu.add)
                    else:
                        nc.vector.tensor_tensor(
                            out=resdst[:, mt, c0:c0 + 512],
                            in0=ln_src[:, mt, c0:c0 + 512], in1=ps[:],
                            op=Alu.add)

        def tap(name, tile_):
            if DBG:
                nc.sync.dma_start(out=dbg[name][:], in_=tile_[:])

        STAGE = int(os.environ.get("KSTAGE", "5"))
        KSUB = set(os.environ.get("KSUB", "").split(","))

        def store_flat(b, tile_):
            of = out[b].rearrange("(p x) d -> p (x d)", p=128)
            nb = min(tile_[:].free_size(), S * DT)
            flat = tile_[:].rearrange("p a b -> p (a b)")[:, 0:nb]
            nc.sync.dma_start(out=of[:, 0:nb], in_=flat)

        # ================= per-batch pipeline =================
        for b in range(BP):
            # ---- embedding + LN1 ----
            ln1T = big.tile([128, DT, S], dt.float32, tag="ln1T", name="ln1T")
            x1T = big.tile([128, DT, S], dt.bfloat16, tag="x1T", name="x1T")
            for half in range(2):
                emb_tiles = []
                for i in range(4):
                    t = half * 4 + i
                    col = b * NT + t
                    emb = ep.tile([128, D], dt.float32, tag="emb", name="emb",
                                  bufs=4)
                    nc.sync.dma_start(out=emb[:],
                                      in_=pos[b, t * 128:(t + 1) * 128, :])
                    if "nogather" not in KSUB:
                        ge = ep.tile([128, D], dt.bfloat16, tag="ge",
                                     name="ge", bufs=2)
                        nc.gpsimd.indirect_dma_start(
                            out=ge[:], out_offset=None, in_=tab_ex[:],
                            in_offset=bass.IndirectOffsetOnAxis(
                                ap=idx_ex_sb[:, col:col + 1], axis=0))
                        gc = ep.tile([128, D], dt.bfloat16, tag="gc",
                                     name="gc", bufs=2)
                        nc.gpsimd.indirect_dma_start(
                            out=gc[:], out_offset=None, in_=tab_cat[:],
                            in_offset=bass.IndirectOffsetOnAxis(
                                ap=idx_cat_sb[:, col:col + 1], axis=0))
                        nc.vector.tensor_tensor(out=emb[:], in0=emb[:],
                                                in1=ge[:], op=Alu.add)
                        nc.vector.tensor_tensor(out=emb[:], in0=emb[:],
                                                in1=gc[:], op=Alu.add)
                    emb_tiles.append(emb)
                if "noln" not in KSUB:
                    ln_inplace(emb_tiles)
                if "notr" not in KSUB:
                    transpose_to_fm(emb_tiles, [ln1T, x1T], t0=half * 4)

            if STAGE <= 1:
                if "nostore" not in KSUB:
                    store_flat(b, ln1T)
                continue
            # ---- en prep ----
            enT = big.tile([128, DT, S], dt.bfloat16, tag="enT", name="enT")
            for half in range(2):
                ent = []
                for i in range(4):
                    t = half * 4 + i
                    et = ep.tile([128, D], dt.float32, tag="emb", name="ent",
                                 bufs=4)
                    nc.sync.dma_start(out=et[:],
                                      in_=en0[b, t * 128:(t + 1) * 128, :])
                    ent.append(et)
                ln_inplace(ent)
                enbf = []
                for i in range(4):
                    eb = sp.tile([128, D], dt.bfloat16, tag="enbf", name="enbf",
                                 bufs=4)
                    nc.vector.tensor_copy(out=eb[:], in_=ent[i][:])
                    enbf.append(eb)
                transpose_to_fm(enbf, [enT], t0=half * 4)

            # ---- attn1 ----
            x1p1 = big.tile([128, DT, S // 2], dt.bfloat16, tag="x1p1",
                            name="x1p1")
            pool_half(x1T, S, x1p1)
            x1p2 = big.tile([128, DT, S // 4], dt.bfloat16, tag="x1p2",
                            name="x1p2")
            pool_half(x1p1, S // 2, x1p2)

            q1T = big.tile([128, DT, S], dt.bfloat16, tag="q1T", name="q1T")
            proj_fm(w["wq1"], x1T, S, q1T, 0, bias["bq1"])
            k1T = big.tile([128, DT, KT], dt.bfloat16, tag="k1T", name="k1T")
            proj_fm(w["wk1"], x1T, S, k1T, 0, bias["bk1"])
            proj_fm(w["wk1"], x1p1, S // 2, k1T, S, bias["bk1"], scale=0.5)
            proj_fm(w["wk1"], x1p2, S // 4, k1T, S + S // 2, bias["bk1"],
                    scale=0.25)
            v65 = big.tile([128, NKT, 8 * 65], dt.bfloat16, tag="v65",
                           name="v65")
            vproj(w["wv1"], [x1T, x1p1, x1p2], v65)

            if STAGE <= 2:
                store_flat(b, k1T)
                continue
            woRhs = big.tile([128, DT, S], dt.bfloat16, tag="woRhs",
                             name="woRhs")
            attention(q1T, k1T, v65, rows["bv1"], woRhs)
            if b == 0:
                tap("d_ln1T", ln1T)
                tap("d_enT", enT)
                tap("d_q1T", q1T)
                tap("d_k1T", k1T)
                tap("d_v65", v65)
                tap("d_woRhs", woRhs)
            res2T = big.tile([128, DT, S], dt.float32, tag="res2T",
                             name="res2T")
            wo_proj_residual(w["wo1"], woRhs, rows["brow1"], res2T, ln1T,
                             bias["g1c"])
            if STAGE <= 3:
                store_flat(b, res2T)
                continue
            if b == 0:
                tap("d_res2T", res2T)
            res2T_bf = big.tile([128, DT, S], dt.bfloat16, tag="v65",
                                name="res2T_bf")
            for mt in range(DT):
                nc.scalar.activation(out=res2T_bf[:, mt, :],
                                     in_=res2T[:, mt, :], func=Act.Copy)

            # ---- attn2 ----
            enp1 = big.tile([128, DT, S // 2], dt.bfloat16, tag="x1p1",
                            name="enp1")
            pool_half(enT, S, enp1)
            enp2 = big.tile([128, DT, S // 4], dt.bfloat16, tag="x1p2",
                            name="enp2")
            pool_half(enp1, S // 2, enp2)

            q2T = big.tile([128, DT, S], dt.bfloat16, tag="q1T", name="q2T")
            proj_fm(w["wq2"], res2T_bf, S, q2T, 0, bias["bq2"])
            k2T = big.tile([128, DT, KT], dt.bfloat16, tag="k1T", name="k2T")
            proj_fm(w["wk2"], enT, S, k2T, 0, bias["bk2"])
            proj_fm(w["wk2"], enp1, S // 2, k2T, S, bias["bk2"], scale=0.5)
            proj_fm(w["wk2"], enp2, S // 4, k2T, S + S // 2, bias["bk2"],
                    scale=0.25)
            v65b = big.tile([128, NKT, 8 * 65], dt.bfloat16, tag="v65",
                            name="v65b")
            vproj(w["wv2"], [enT, enp1, enp2], v65b)

            woRhs2 = big.tile([128, DT, S], dt.bfloat16, tag="woRhs",
                              name="woRhs2")
            attention(q2T, k2T, v65b, rows["bv2"], woRhs2)
            wo_proj_residual(w["wo2"], woRhs2, rows["brow2"], res2T, res2T,
                             None)
            res3T = res2T
            if b == 0:
                tap("d_res3T", res3T)

            if STAGE <= 4:
                store_flat(b, res3T)
                continue
            # ---- LN3 (transpose to tm, LN, transpose back) ----
            x3T = big.tile([128, DT, S], dt.float32, tag="ln1T", name="x3T")
            x3T_bf = big.tile([128, DT, S], dt.bfloat16, tag="enT",
                              name="x3T_bf")
            for half in range(2):
                r3 = []
                for i in range(4):
                    t = half * 4 + i
                    rt = sp.tile([128, D], dt.float32, tag="res3tm",
                                 name="res3tm", bufs=4)
                    for d_ in range(DT):
                        ptr = pp_tr.tile([128, 128], dt.float32, tag="tr",
                                         name="tr")
                        nc.tensor.transpose(
                            out=ptr[:],
                            in_=res3T[:, d_, t * 128:(t + 1) * 128],
                            identity=id_f[:])
                        nc.vector.tensor_copy(
                            out=rt[:, d_ * 128:(d_ + 1) * 128], in_=ptr[:])
                    r3.append(rt)
                ln_inplace(r3)
                transpose_to_fm(r3, [x3T, x3T_bf], t0=half * 4)

            # ---- FFN ----
            if b == 0:
                tap("d_x3T", x3T)
            hT = big.tile([128, DT, S], dt.bfloat16, tag="q1T", name="hT")
            proj_fm(w["fw1"], x3T_bf, S, hT, 0, bias["fb1"], func=Act.Relu)
            if b == 0:
                tap("d_hT", hT)
            finT = big.tile([128, DT, S], dt.float32, tag="k1T", name="finT")
            for mt in range(DT):
                for c0 in range(0, S, 512):
                    ps = pp.tile([128, 512], dt.float32, tag="proj",
                                 name="proj")
                    for kt in range(DT):
                        nc.tensor.matmul(
                            out=ps[:],
                            lhsT=w["fw2"][:, kt, mt * 128:(mt + 1) * 128],
                            rhs=hT[:, kt, c0:c0 + 512],
                            start=(kt == 0), stop=False)
                    nc.tensor.matmul(
                        out=ps[:],
                        lhsT=rows["browF"][0:1, mt * 128:(mt + 1) * 128],
                        rhs=ones_row[0:1, c0:c0 + 512], start=False, stop=True)
                    nc.vector.scalar_tensor_tensor(
                        out=finT[:, mt, c0:c0 + 512],
                        in0=x3T[:, mt, c0:c0 + 512],
                        scalar=bias["g3c"][:, mt:mt + 1], in1=ps[:],
                        op0=Alu.mult, op1=Alu.add)

            if b == 0:
                tap("d_finT", finT)
            # ---- final transpose + store ----
            for t in range(NT):
                ft = sp.tile([128, D], dt.float32, tag="fin_tm", name="fin_tm",
                             bufs=2)
                for d_ in range(DT):
                    ptr = pp_tr.tile([128, 128], dt.float32, tag="tr",
                                     name="tr")
                    nc.tensor.transpose(
                        out=ptr[:], in_=finT[:, d_, t * 128:(t + 1) * 128],
                        identity=id_f[:])
                    nc.vector.tensor_copy(out=ft[:, d_ * 128:(d_ + 1) * 128],
                                          in_=ptr[:])
                nc.sync.dma_start(out=out[b, t * 128:(t + 1) * 128, :],
                                  in_=ft[:])

    nc.finalize()
    return nc


def _prep_inputs(inputs):
    inp = {k: np.asarray(v) for k, v in inputs.items()}
    f32 = lambda x: np.ascontiguousarray(np.asarray(x, F32))
    bf = lambda x: np.ascontiguousarray(np.asarray(np.asarray(x, F32), BF))

    E_ex2 = inp["E_ex"][:2 * NUM_Q].astype(F32) \
        + np.repeat(inp["E_res"][:2].astype(F32), NUM_Q, 0)
    tab_ex = bf(E_ex2)
    tab_cat = bf(inp["E_cat"][:2 * NUM_C])
    idx_ex = (inp["in_ex"].astype(np.int64)
              + NUM_Q * inp["in_res"].astype(np.int64)).astype(np.int32)
    idx_cat = (inp["in_cat"].astype(np.int64)
               + NUM_C * inp["in_res"].astype(np.int64)).astype(np.int32)

    g1, be1, g2, be2, g3, be3 = (inp[k].astype(F32) for k in
                                 ["g1", "be1", "g2", "be2", "g3", "be3"])

    def wlayout(a):
        return np.ascontiguousarray(
            np.asarray(np.asarray(a, F32), BF).reshape(DT, 128, D)
            .transpose(1, 0, 2))

    def blayout(a):
        return np.ascontiguousarray(np.asarray(a, F32).reshape(DT, 128).T)

    wmap = {
        "wq1": wlayout(g1[:, None] * inp["Wq1"] / 8.0),
        "wk1": wlayout(g1[:, None] * inp["Wk1"]),
        "wv1": wlayout(g1[:, None] * inp["Wv1"]),
        "wo1": wlayout(inp["Wo1"]),
        "wq2": wlayout(inp["Wq2"] / 8.0),
        "wk2": wlayout(g2[:, None] * inp["Wk2"]),
        "wv2": wlayout(g2[:, None] * inp["Wv2"]),
        "wo2": wlayout(inp["Wo2"]),
        "fw1": wlayout(g3[:, None] * inp["fW1"]),
        "fw2": wlayout(inp["fW2"]),
    }
    bmap = {
        "bq1": blayout(inp["bq1"] / 8.0 + be1 @ inp["Wq1"] / 8.0),
        "bk1": blayout(inp["bk1"] + be1 @ inp["Wk1"]),
        "bq2": blayout(inp["bq2"] / 8.0),
        "bk2": blayout(inp["bk2"] + be2 @ inp["Wk2"]),
        "fb1": blayout(inp["fb1"] + be3 @ inp["fW1"]),
        "g1c": blayout(g1),
        "g3c": blayout(g3),
    }
    rmap = {
        "bv1": bf(inp["bv1"] + be1 @ inp["Wv1"])[None, :],
        "bv2": bf(inp["bv2"] + be2 @ inp["Wv2"])[None, :],
        "brow1": bf(inp["bo1"] + be1)[None, :],
        "brow2": bf(inp["bo2"])[None, :],
        "browF": bf(inp["fb2"] + be3)[None, :],
    }
    m0, m1, m2 = _build_strips()
    common = dict(tab_ex=tab_ex, tab_cat=tab_cat, mask0=bf(m0), mask1=bf(m1),
                  mask2=bf(m2), **wmap, **bmap, **rmap)

    in_maps = []
    for c in range(NCORES):
        bs = slice(c * BP, (c + 1) * BP)
        m = dict(common)
        m["idx_ex"] = np.ascontiguousarray(idx_ex[bs].reshape(BP * NT, 128).T)
        m["idx_cat"] = np.ascontiguousarray(idx_cat[bs].reshape(BP * NT, 128).T)
        m["pos"] = f32(inp["in_pos"][bs])
        m["en0"] = f32(inp["en_out"][bs])
        in_maps.append(m)
    return in_maps


def kernel(**inputs):
    from concourse.bass_utils import run_bass_kernel_spmd

    if "nc" not in _prog_cache:
        _prog_cache["nc"] = build_program()
    nc = _prog_cache["nc"]
    in_maps = _prep_inputs(inputs)
    res = run_bass_kernel_spmd(nc, in_maps, list(range(NCORES)))
    _prog_cache["last_result"] = res
    outs = [res.results[c]["out"] for c in range(NCORES)]
    return np.concatenate(outs, 0)


if __name__ == "__main__":
    nc = build_program()
    n_inst = sum(len(bb.instructions) for bb in nc.main_func.blocks)
    print(f"program built: {n_inst} instructions")
